# revision 2
# baseline (speedup 1.0000x reference)
import sys

sys.path.insert(0, "/opt/trn_rl_repo")

import numpy as np

import concourse.bass as bass  # noqa: F401
import concourse.tile as tile
from concourse import bacc, mybir
from concourse.bass_utils import run_bass_kernel_spmd

# Problem shapes (hardcoded per contract).
N, D = 16384, 2048
NCORES = 8
NSHARD = N // NCORES  # 2048 rows per core
HALF_LOG_2PI = 0.5 * float(np.log(2.0 * np.pi))

P = 128  # SBUF partitions
KC = D // P  # 16 k-chunks of 128 features
NBLK = 512  # n-columns per transposed block (= 1 PSUM bank of f32)
TB = NBLK // P  # 4 row-tiles per block
BLOCKS = NSHARD // NBLK  # 4 blocks per core

_PROGRAM = None
LAST_RESULT = None  # BassKernelResults of the most recent run (for test harness)


# Tuned on HW (axon trn2, 8 cores). Steady-state per pass == the pure-DMA
# floor within noise (measured 2026-08-10: full kernel 47.0-48.7 us vs
# dma-only 46.2-48.8 us in same-window interleaved A/Bs) — i.e. the kernel
# sits AT the HBM roofline: 16.78 MB/core / ~358 GB/s-per-NC (716 GB/s per
# stack shared by an NC pair, all 8 cores streaming) = 46.9 us. Re-tested
# and rejected: 2- and 3-ring DMA splits (SWDGE + sync/scalar HWDGE —
# HBM-capped, no gain), n_dve=0 (all squares on ACT; tied with n_dve=4),
# pack_accs, store_eng=sync. Absolute numbers drift 47-55 us with terminal
# conditions; only interleaved same-round deltas are meaningful.
DEFAULT_CFG = dict(
    # 8 big DMAs/pass (4x3MB + 4x1MB): fewer/bigger transfers keep the
    # single SWDGE ring at ~425 GB/s (beat (512,1024,512) by ~3us).
    slabs=(1536, 512),
    pst_bufs=4,
    up_bufs=6,
    n_dve=4,  # balances DVE (16 chunks) against ACT (48 chunks)
    use_fp16=True,
    mm_transpose=True,
    two_phase=True,
    combine_dma=True,
    ring_pingpong=True,  # (ignored for loads under cast_dma)
    # f32->fp16 cast inside the SWDGE load DMA: removes the DVE convert
    # stage entirely and halves SBUF-side DMA write traffic. Single
    # qPoolDynamic ring sustains ~415 GB/s DRAM reads (measured).
    cast_dma=True,
    store_eng="scalar",
)


def _build_program(
    repeat=1,
    loop_repeat=0,
    xp_bufs=None,
    pst_bufs=4,
    up_bufs=6,
    slabs=(512, 1024, 512),
    n_dve=6,
    dma_only=False,
    two_phase=True,
    use_fp16=False,
    cvt_bufs=None,
    stage=4,
    mm_transpose=False,
    combine_dma=True,
    pack_accs=False,
    ring_pingpong=False,
    ring_order=None,  # tuple of engine names for load-DMA rotation
    store_eng="gpsimd",  # engine whose queue carries the out stores
    cast_dma=False,  # f32->fp16 cast inside the (SWDGE) load DMA
    sync_slabs=(),  # under cast_dma: slabs loaded f32 via ring_order + DVE
    head_split=0,  # split block0/slab0 load: first piece this many cols
    tail_split=0,  # split last-block/last-slab load: last piece this wide
):
    """stage: 0=dma only, 1=+convert, 2=+transpose, 3=+act/dve square,
    4=full (reduce matmul + gamma + store)."""
    if dma_only:
        stage = 0
    """out[n] = gamma - sum_k (p[k]*x[n,k] + q[k])**2, computed as:
    PE-transpose x into [k, n] layout, one fused ACT Square(p*x+q) pass
    (per-partition scale/bias = per-feature), then a [-1]*u matmul reduce
    over k accumulated in PSUM.

    repeat>1 re-runs the whole pass inside one NEFF (for differential
    HW timing only; results identical).
    """
    nc = bacc.Bacc(
        "TRN2",
        target_bir_lowering=False,
        debug=False,
        enable_asserts=False,
        num_devices=NCORES,
    )
    f32 = mybir.dt.float32
    bf16 = mybir.dt.bfloat16
    fp16 = mybir.dt.float16
    if mm_transpose:
        # transpose = regular matmul xh.T @ I (fp16, 1 cyc/row, no PE
        # transpose-mode toggling); regular matmuls must write fp32 PSUM
        use_fp16 = True
    tdt = f32 if (mm_transpose or not use_fp16) else fp16  # PSUM dtype

    x_ap = nc.dram_tensor("x", [NSHARD, D], f32, kind="ExternalInput").ap()
    # One packed coef tensor: cols [0,KC)=p, [KC,2KC)=q, [2KC]=gamma(row 0),
    # [2KC+1, 2KC+1+P)=identity. Single DMA instead of five.
    CW = 2 * KC + 1 + P
    cf_ap = nc.dram_tensor("coefs", [P, CW], f32, kind="ExternalInput").ap()
    idh_ap = None
    if use_fp16:
        idh_ap = nc.dram_tensor("identh", [P, P], fp16, kind="ExternalInput").ap()
    out_ap = nc.dram_tensor("out", [1, NSHARD], f32, kind="ExternalOutput").ap()

    assert sum(slabs) == D and all(s % P == 0 for s in slabs)
    nslab = len(slabs)
    slab_off = [sum(slabs[:i]) for i in range(nslab)]  # k-col start of slab i
    chunk_slab = []  # chunk index -> (slab idx, chunk-within-slab)
    for i, s in enumerate(slabs):
        for j in range(s // P):
            chunk_slab.append((i, j))
    if xp_bufs is None:
        # with the fp16 convert stage, raw f32 tiles are transient
        xp_bufs = 2 if use_fp16 else BLOCKS
    if cvt_bufs is None:
        cvt_bufs = BLOCKS  # converted tiles: whole shard resident

    with tile.TileContext(nc) as tc:
        with (
            tc.tile_pool(name="coef", bufs=1) as coef,
            tc.tile_pool(name="xp", bufs=xp_bufs) as xp,
            tc.tile_pool(name="xc", bufs=cvt_bufs) as xc,
            tc.tile_pool(name="up", bufs=up_bufs) as up,
            tc.tile_pool(name="obuf", bufs=2) as obuf,
            tc.tile_pool(name="pst", bufs=pst_bufs, space="PSUM") as pst,
            tc.tile_pool(
                name="psa",
                bufs=1 if pack_accs else (BLOCKS if two_phase else 2),
                space="PSUM",
            ) as psa,
        ):
            # Constant load goes on ACT's DMA ring so the SP ring is free to
            # start streaming x immediately (the two rings are independent).
            cf_sb = coef.tile([P, CW], f32)
            nc.scalar.dma_start(cf_sb[:], cf_ap[:, :])
            p_sb = cf_sb[:, 0:KC]
            q_sb = cf_sb[:, KC : 2 * KC]
            g_sb = cf_sb[:, 2 * KC : 2 * KC + 1]
            id_sb = cf_sb[:, 2 * KC + 1 : 2 * KC + 1 + P]
            no_sb = coef.tile([P, 1], bf16)
            nc.vector.memset(no_sb[:], -1.0)
            if use_fp16:
                idh_sb = coef.tile([P, P], fp16)
                nc.scalar.dma_start(idh_sb[:], idh_ap[:, :])
                id_t = idh_sb
            else:
                id_t = id_sb

            # DVE takes n_dve of the KC chunks per block, preferring the
            # last slab's chunks (they're on the post-DMA critical tail).
            dve_order = [9, 11, 13, 15, 1, 3, 5, 7, 8, 10, 12, 14, 0, 2, 4, 6]
            dve_set = set(dve_order[:n_dve])

            def do_chunk(b, c, xs, acc):
                if stage < 2:
                    return
                s, ck = chunk_slab[c]
                pt = pst.tile([P, NBLK], tdt)
                for t in range(TB):
                    if mm_transpose:
                        nc.tensor.matmul(
                            pt[:, t * P : (t + 1) * P],
                            xs[t][s][:, ck * P : (ck + 1) * P],
                            id_t,
                            start=True,
                            stop=True,
                        )
                    else:
                        nc.tensor.transpose(
                            pt[:, t * P : (t + 1) * P],
                            xs[t][s][:, ck * P : (ck + 1) * P],
                            id_t,
                        )
                if stage < 3:
                    return
                if c in dve_set:
                    # fp16 u: |u| <= ~5e4 even for worst-case deg/std, and
                    # fp16's 10-bit mantissa quarters the rounding noise of
                    # bf16 before the square.
                    u = up.tile([P, NBLK], fp16)
                    nc.vector.tensor_scalar(
                        u[:],
                        pt[:],
                        p_sb[:, c : c + 1],
                        q_sb[:, c : c + 1],
                        mybir.AluOpType.mult,
                        mybir.AluOpType.add,
                    )
                    w = up.tile([P, NBLK], bf16)
                    nc.vector.tensor_mul(w[:], u[:], u[:])
                else:
                    w = up.tile([P, NBLK], bf16)
                    nc.scalar.activation(
                        w[:],
                        pt[:],
                        mybir.ActivationFunctionType.Square,
                        bias=q_sb[:, c : c + 1],
                        scale=p_sb[:, c : c + 1],
                    )
                if stage < 4:
                    return
                nc.tensor.matmul(
                    acc[:],
                    no_sb[:],
                    w[:],
                    start=(c == 0),
                    stop=(c == KC - 1),
                )

            dma_seq = [0]

            eng_by_name = {
                "sync": nc.sync,
                "gpsimd": nc.gpsimd,
                "scalar": nc.scalar,
            }

            def load_slab(b, s, xs):
                # One DMA per (block, slab) carries all TB row-tiles: the
                # DRAM side is read as [(t p) c -> p (t c)], so partition p
                # receives rows r0+t*128+p for t=0..TB-1. Fewer, bigger DMAs
                # (~0.25us fixed cost each on the serial ring).
                eng = nc.sync
                if ring_order:
                    eng = eng_by_name[ring_order[dma_seq[0] % len(ring_order)]]
                    dma_seq[0] += 1
                elif ring_pingpong:
                    # rotate across DMA queues (SP HWDGE, GPSIMD SWDGE, and
                    # optionally ACT HWDGE) so consecutive transfers hide
                    # each other's issue/completion gap
                    nq = int(ring_pingpong) + 1
                    eng = [nc.sync, nc.gpsimd, nc.scalar][dma_seq[0] % nq]
                    dma_seq[0] += 1
                r0 = b * TB * P
                if cast_dma and s not in sync_slabs:
                    # SWDGE (gpsimd) casts f32->fp16 in flight: no f32
                    # staging tile, no DVE convert pass.
                    xh = xc.tile([P, TB * slabs[s]], fp16, tag=f"xc{s}")
                    # Optional piecewise load: a small first piece lets
                    # compute start early on the cold pass (head), a small
                    # last piece shrinks the post-last-byte tail.
                    pieces = [slabs[s]]
                    if head_split and b == 0 and s == 0 and head_split < slabs[s]:
                        pieces = [head_split, slabs[s] - head_split]
                    if (
                        tail_split
                        and b == BLOCKS - 1
                        and s == nslab - 1
                        and tail_split < slabs[s]
                    ):
                        pieces = [slabs[s] - tail_split, tail_split]
                    c0 = 0
                    for w in pieces:
                        src = x_ap[
                            r0 : r0 + TB * P,
                            slab_off[s] + c0 : slab_off[s] + c0 + w,
                        ].rearrange("(t p) c -> p t c", p=P)
                        dst = xh[:].rearrange("p (t c) -> p t c", t=TB)[
                            :, :, c0 : c0 + w
                        ]
                        nc.gpsimd.dma_start(dst, src)
                        c0 += w
                    big = xh
                    for t in range(TB):
                        xs[t][s] = big[:, t * slabs[s] : (t + 1) * slabs[s]]
                    return
                xt = xp.tile([P, TB * slabs[s]], f32, tag=f"x{s}")
                if combine_dma:
                    src = x_ap[
                        r0 : r0 + TB * P, slab_off[s] : slab_off[s] + slabs[s]
                    ].rearrange("(t p) c -> p t c", p=P)
                    dst = xt[:].rearrange("p (t c) -> p t c", t=TB)
                    eng.dma_start(dst, src)
                else:
                    for t in range(TB):
                        eng.dma_start(
                            xt[:, t * slabs[s] : (t + 1) * slabs[s]],
                            x_ap[
                                r0 + t * P : r0 + (t + 1) * P,
                                slab_off[s] : slab_off[s] + slabs[s],
                            ],
                        )
                if use_fp16 and stage >= 1:
                    # fp16 copy on DVE (2x single-src mode) so the PE
                    # transpose runs at 1 cyc/row instead of fp32's 2.
                    xh = xc.tile([P, TB * slabs[s]], fp16, tag=f"xc{s}")
                    nc.vector.tensor_copy(xh[:], xt[:])
                    big = xh
                else:
                    big = xt
                for t in range(TB):
                    xs[t][s] = big[:, t * slabs[s] : (t + 1) * slabs[s]]

            def finish_block(b, acc):
                ob = obuf.tile([1, NBLK], f32)
                if stage < 4:
                    nc.vector.memset(ob[:], 0.0)
                else:
                    nc.vector.tensor_scalar_add(ob[:], acc[:], g_sb[0:1, 0:1])
                # out stores ride a configurable queue (they depend on
                # compute; on the load rings they could stall later x loads).
                eng_by_name[store_eng].dma_start(
                    out_ap[0:1, b * NBLK : (b + 1) * NBLK], ob[:]
                )

            def one_pass():
                if not two_phase:
                    for b in range(BLOCKS):
                        xs = [[None] * nslab for _ in range(TB)]
                        for s in range(nslab):
                            load_slab(b, s, xs)
                        acc = psa.tile([1, NBLK], f32)
                        for c in range(KC):
                            do_chunk(b, c, xs, acc)
                        finish_block(b, acc)
                    return
                # Two-phase schedule: every block's slab-0 work first (all
                # BLOCKS accumulators stay live in PSUM), then slab-1 per
                # block. The post-last-DMA tail is one block's last-slab
                # chunks only.
                xs_all = [[[None] * nslab for _ in range(TB)] for _ in range(BLOCKS)]
                accs = [None] * BLOCKS
                if pack_accs:
                    # three block accumulators share one PSUM bank at
                    # partitions 0/32/64 (the legal AP base partitions);
                    # the fourth gets its own bank. Frees 2 banks for pst.
                    acc3 = psa.tile([128, NBLK], f32, tag="acc3")
                    accl = psa.tile([1, NBLK], f32, tag="accl")
                    accs = [acc3[32 * b : 32 * b + 1, :] for b in range(3)] + [accl]
                first_chunks = [c for c in range(KC) if chunk_slab[c][0] < nslab - 1]
                last_chunks = [c for c in range(KC) if chunk_slab[c][0] == nslab - 1]
                for b in range(BLOCKS):
                    for s in range(nslab - 1):
                        load_slab(b, s, xs_all[b])
                    if not pack_accs:
                        acc = psa.tile([1, NBLK], f32)
                        accs[b] = acc
                    for c in first_chunks:
                        do_chunk(b, c, xs_all[b], accs[b])
                s_last = nslab - 1
                for b in range(BLOCKS):
                    load_slab(b, s_last, xs_all[b])
                    for c in last_chunks:
                        do_chunk(b, c, xs_all[b], accs[b])
                    ob = obuf.tile([1, NBLK], f32)
                    if stage < 4:
                        nc.vector.memset(ob[:], 0.0)
                    else:
                        nc.vector.tensor_scalar_add(ob[:], accs[b][:], g_sb[0:1, 0:1])
                    # out stores ride a configurable queue (they depend on
                    # compute; on the load rings they could stall x loads).
                    eng_by_name[store_eng].dma_start(
                        out_ap[0:1, b * NBLK : (b + 1) * NBLK], ob[:]
                    )

            if loop_repeat:
                # HW loop wrapper for differential timing: each iteration is
                # a full cold-ish pass (the back-edge is an all-engine drain).
                with tc.For_i(
                    0, loop_repeat, 1, hint_engines=(mybir.EngineType.PE,)
                ):
                    one_pass()
            else:
                for _rep in range(repeat):
                    one_pass()

    nc.compile()
    return nc


def make_in_maps(x, raw_params, edges):
    x = np.ascontiguousarray(np.asarray(x, dtype=np.float32))
    raw_params = np.asarray(raw_params, dtype=np.float64)
    edges = np.asarray(edges)
    assert x.shape == (N, D), x.shape

    # Tiny host-side coefficient math (O(D); the O(N*D) pass runs on device).
    means = np.tanh(raw_params[:D]) * 2.0
    stds = np.logaddexp(0.0, raw_params[D:]) + 1e-6  # softplus + eps
    deg = np.zeros(D, dtype=np.float64)
    np.add.at(deg, edges.reshape(-1), 1.0)
    p = np.sqrt(0.5 * deg) / stds
    q = -means * p
    gamma = float(-np.sum(deg * (np.log(stds) + HALF_LOG_2PI)))

    p2d = p.reshape(KC, P).T.astype(np.float32)
    q2d = q.reshape(KC, P).T.astype(np.float32)
    coefs = np.zeros((P, 2 * KC + 1 + P), dtype=np.float32)
    coefs[:, 0:KC] = p2d
    coefs[:, KC : 2 * KC] = q2d
    coefs[:, 2 * KC] = gamma
    coefs[:, 2 * KC + 1 :] = np.eye(P, dtype=np.float32)

    in_maps = []
    for c in range(NCORES):
        shard = x[c * NSHARD : (c + 1) * NSHARD]
        m = {"x": shard, "coefs": coefs}
        if DEFAULT_CFG.get("use_fp16"):
            m["identh"] = np.eye(P, dtype=np.float16)
        in_maps.append(m)
    return in_maps


def kernel(x, raw_params, edges, _trace=False):
    global _PROGRAM, LAST_RESULT
    in_maps = make_in_maps(x, raw_params, edges)

    if _PROGRAM is None:
        _PROGRAM = _build_program(**DEFAULT_CFG)
    nc = _PROGRAM

    LAST_RESULT = run_bass_kernel_spmd(
        nc, in_maps, core_ids=list(range(NCORES)), trace=_trace
    )
    out = np.concatenate(
        [LAST_RESULT.results[c]["out"].reshape(-1) for c in range(NCORES)]
    )
    return out.astype(np.float32)



# revision 4
# speedup vs baseline: 1.0113x; 1.0113x over previous
import sys

sys.path.insert(0, "/opt/trn_rl_repo")

import numpy as np

import concourse.bass as bass  # noqa: F401
import concourse.tile as tile
from concourse import bacc, mybir
from concourse.bass_utils import run_bass_kernel_spmd

# Problem shapes (hardcoded per contract).
N, D = 16384, 2048
NCORES = 8
NSHARD = N // NCORES  # 2048 rows per core
HALF_LOG_2PI = 0.5 * float(np.log(2.0 * np.pi))

P = 128  # SBUF partitions
KC = D // P  # 16 k-chunks of 128 features
NBLK = 512  # n-columns per transposed block (= 1 PSUM bank of f32)
TB = NBLK // P  # 4 row-tiles per block
BLOCKS = NSHARD // NBLK  # 4 blocks per core

_PROGRAM = None
LAST_RESULT = None  # BassKernelResults of the most recent run (for test harness)


# Tuned on HW (axon trn2, 8 cores). Steady-state per pass == the pure-DMA
# floor within noise (measured 2026-08-10: full kernel 47.0-48.7 us vs
# dma-only 46.2-48.8 us in same-window interleaved A/Bs) — i.e. the kernel
# sits AT the HBM roofline: 16.78 MB/core / ~358 GB/s-per-NC (716 GB/s per
# stack shared by an NC pair, all 8 cores streaming) = 46.9 us. Re-tested
# and rejected: 2- and 3-ring DMA splits (SWDGE + sync/scalar HWDGE —
# HBM-capped, no gain), n_dve=0 (all squares on ACT; tied with n_dve=4),
# pack_accs, store_eng=sync. Absolute numbers drift 47-55 us with terminal
# conditions; only interleaved same-round deltas are meaningful.
DEFAULT_CFG = dict(
    # 16 uniform 1MB DMAs/pass, each partition reading ONE fully-contiguous
    # 8KB DRAM row (slabs=(2048,) + combine_dma=False): beat the two-phase
    # (1536,512) chunked "(t p) c" pattern by ~0.6us/pass (4/4 interleaved
    # rounds), landing within ~0.2us of the same-round pure-DMA floor.
    slabs=(2048,),
    pst_bufs=4,
    up_bufs=6,
    n_dve=4,  # balances DVE (16 chunks) against ACT (48 chunks)
    use_fp16=True,
    mm_transpose=True,
    two_phase=True,  # degenerates to per-block load+compute with one slab
    combine_dma=False,
    ring_pingpong=True,  # (ignored for loads under cast_dma)
    # f32->fp16 cast inside the SWDGE load DMA: removes the DVE convert
    # stage entirely and halves SBUF-side DMA write traffic. Single
    # qPoolDynamic ring sustains ~415 GB/s DRAM reads (measured).
    cast_dma=True,
    store_eng="scalar",
)


def _build_program(
    repeat=1,
    loop_repeat=0,
    xp_bufs=None,
    pst_bufs=4,
    up_bufs=6,
    slabs=(512, 1024, 512),
    n_dve=6,
    dma_only=False,
    two_phase=True,
    use_fp16=False,
    cvt_bufs=None,
    stage=4,
    mm_transpose=False,
    combine_dma=True,
    pack_accs=False,
    ring_pingpong=False,
    ring_order=None,  # tuple of engine names for load-DMA rotation
    store_eng="gpsimd",  # engine whose queue carries the out stores
    cast_dma=False,  # f32->fp16 cast inside the (SWDGE) load DMA
    sync_slabs=(),  # under cast_dma: slabs loaded f32 via ring_order + DVE
    head_split=0,  # split block0/slab0 load: first piece this many cols
    tail_split=0,  # split last-block/last-slab load: last piece this wide
):
    """stage: 0=dma only, 1=+convert, 2=+transpose, 3=+act/dve square,
    4=full (reduce matmul + gamma + store)."""
    if dma_only:
        stage = 0
    """out[n] = gamma - sum_k (p[k]*x[n,k] + q[k])**2, computed as:
    PE-transpose x into [k, n] layout, one fused ACT Square(p*x+q) pass
    (per-partition scale/bias = per-feature), then a [-1]*u matmul reduce
    over k accumulated in PSUM.

    repeat>1 re-runs the whole pass inside one NEFF (for differential
    HW timing only; results identical).
    """
    nc = bacc.Bacc(
        "TRN2",
        target_bir_lowering=False,
        debug=False,
        enable_asserts=False,
        num_devices=NCORES,
    )
    f32 = mybir.dt.float32
    bf16 = mybir.dt.bfloat16
    fp16 = mybir.dt.float16
    if mm_transpose:
        # transpose = regular matmul xh.T @ I (fp16, 1 cyc/row, no PE
        # transpose-mode toggling); regular matmuls must write fp32 PSUM
        use_fp16 = True
    tdt = f32 if (mm_transpose or not use_fp16) else fp16  # PSUM dtype

    x_ap = nc.dram_tensor("x", [NSHARD, D], f32, kind="ExternalInput").ap()
    # One packed coef tensor: cols [0,KC)=p, [KC,2KC)=q, [2KC]=gamma(row 0),
    # [2KC+1, 2KC+1+P)=identity. Single DMA instead of five.
    CW = 2 * KC + 1 + P
    cf_ap = nc.dram_tensor("coefs", [P, CW], f32, kind="ExternalInput").ap()
    idh_ap = None
    if use_fp16:
        idh_ap = nc.dram_tensor("identh", [P, P], fp16, kind="ExternalInput").ap()
    out_ap = nc.dram_tensor("out", [1, NSHARD], f32, kind="ExternalOutput").ap()

    assert sum(slabs) == D and all(s % P == 0 for s in slabs)
    nslab = len(slabs)
    slab_off = [sum(slabs[:i]) for i in range(nslab)]  # k-col start of slab i
    chunk_slab = []  # chunk index -> (slab idx, chunk-within-slab)
    for i, s in enumerate(slabs):
        for j in range(s // P):
            chunk_slab.append((i, j))
    if xp_bufs is None:
        # with the fp16 convert stage, raw f32 tiles are transient
        xp_bufs = 2 if use_fp16 else BLOCKS
    if cvt_bufs is None:
        cvt_bufs = BLOCKS  # converted tiles: whole shard resident

    with tile.TileContext(nc) as tc:
        with (
            tc.tile_pool(name="coef", bufs=1) as coef,
            tc.tile_pool(name="xp", bufs=xp_bufs) as xp,
            tc.tile_pool(name="xc", bufs=cvt_bufs) as xc,
            tc.tile_pool(name="up", bufs=up_bufs) as up,
            tc.tile_pool(name="obuf", bufs=2) as obuf,
            tc.tile_pool(name="pst", bufs=pst_bufs, space="PSUM") as pst,
            tc.tile_pool(
                name="psa",
                bufs=1 if pack_accs else (BLOCKS if two_phase else 2),
                space="PSUM",
            ) as psa,
        ):
            # Constant load goes on ACT's DMA ring so the SP ring is free to
            # start streaming x immediately (the two rings are independent).
            cf_sb = coef.tile([P, CW], f32)
            nc.scalar.dma_start(cf_sb[:], cf_ap[:, :])
            p_sb = cf_sb[:, 0:KC]
            q_sb = cf_sb[:, KC : 2 * KC]
            g_sb = cf_sb[:, 2 * KC : 2 * KC + 1]
            id_sb = cf_sb[:, 2 * KC + 1 : 2 * KC + 1 + P]
            no_sb = coef.tile([P, 1], bf16)
            nc.vector.memset(no_sb[:], -1.0)
            if use_fp16:
                idh_sb = coef.tile([P, P], fp16)
                nc.scalar.dma_start(idh_sb[:], idh_ap[:, :])
                id_t = idh_sb
            else:
                id_t = id_sb

            # DVE takes n_dve of the KC chunks per block, preferring the
            # last slab's chunks (they're on the post-DMA critical tail).
            dve_order = [9, 11, 13, 15, 1, 3, 5, 7, 8, 10, 12, 14, 0, 2, 4, 6]
            dve_set = set(dve_order[:n_dve])

            def do_chunk(b, c, xs, acc):
                if stage < 2:
                    return
                s, ck = chunk_slab[c]
                pt = pst.tile([P, NBLK], tdt)
                for t in range(TB):
                    if mm_transpose:
                        nc.tensor.matmul(
                            pt[:, t * P : (t + 1) * P],
                            xs[t][s][:, ck * P : (ck + 1) * P],
                            id_t,
                            start=True,
                            stop=True,
                        )
                    else:
                        nc.tensor.transpose(
                            pt[:, t * P : (t + 1) * P],
                            xs[t][s][:, ck * P : (ck + 1) * P],
                            id_t,
                        )
                if stage < 3:
                    return
                if c in dve_set:
                    # fp16 u: |u| <= ~5e4 even for worst-case deg/std, and
                    # fp16's 10-bit mantissa quarters the rounding noise of
                    # bf16 before the square.
                    u = up.tile([P, NBLK], fp16)
                    nc.vector.tensor_scalar(
                        u[:],
                        pt[:],
                        p_sb[:, c : c + 1],
                        q_sb[:, c : c + 1],
                        mybir.AluOpType.mult,
                        mybir.AluOpType.add,
                    )
                    w = up.tile([P, NBLK], bf16)
                    nc.vector.tensor_mul(w[:], u[:], u[:])
                else:
                    w = up.tile([P, NBLK], bf16)
                    nc.scalar.activation(
                        w[:],
                        pt[:],
                        mybir.ActivationFunctionType.Square,
                        bias=q_sb[:, c : c + 1],
                        scale=p_sb[:, c : c + 1],
                    )
                if stage < 4:
                    return
                nc.tensor.matmul(
                    acc[:],
                    no_sb[:],
                    w[:],
                    start=(c == 0),
                    stop=(c == KC - 1),
                )

            dma_seq = [0]

            eng_by_name = {
                "sync": nc.sync,
                "gpsimd": nc.gpsimd,
                "scalar": nc.scalar,
            }

            def load_slab(b, s, xs):
                # One DMA per (block, slab) carries all TB row-tiles: the
                # DRAM side is read as [(t p) c -> p (t c)], so partition p
                # receives rows r0+t*128+p for t=0..TB-1. Fewer, bigger DMAs
                # (~0.25us fixed cost each on the serial ring).
                eng = nc.sync
                if ring_order:
                    eng = eng_by_name[ring_order[dma_seq[0] % len(ring_order)]]
                    dma_seq[0] += 1
                elif ring_pingpong:
                    # rotate across DMA queues (SP HWDGE, GPSIMD SWDGE, and
                    # optionally ACT HWDGE) so consecutive transfers hide
                    # each other's issue/completion gap
                    nq = int(ring_pingpong) + 1
                    eng = [nc.sync, nc.gpsimd, nc.scalar][dma_seq[0] % nq]
                    dma_seq[0] += 1
                r0 = b * TB * P
                if cast_dma and s not in sync_slabs:
                    # SWDGE (gpsimd) casts f32->fp16 in flight: no f32
                    # staging tile, no DVE convert pass.
                    xh = xc.tile([P, TB * slabs[s]], fp16, tag=f"xc{s}")
                    if not combine_dma:
                        # per-row-tile loads: partition p <- row r0+t*128+p,
                        # fully contiguous 4*slabs[s] B per partition in DRAM
                        for t in range(TB):
                            nc.gpsimd.dma_start(
                                xh[:, t * slabs[s] : (t + 1) * slabs[s]],
                                x_ap[
                                    r0 + t * P : r0 + (t + 1) * P,
                                    slab_off[s] : slab_off[s] + slabs[s],
                                ],
                            )
                        for t in range(TB):
                            xs[t][s] = xh[:, t * slabs[s] : (t + 1) * slabs[s]]
                        return
                    # Optional piecewise load: a small first piece lets
                    # compute start early on the cold pass (head), a small
                    # last piece shrinks the post-last-byte tail.
                    pieces = [slabs[s]]
                    if head_split and b == 0 and s == 0 and head_split < slabs[s]:
                        pieces = [head_split, slabs[s] - head_split]
                    if (
                        tail_split
                        and b == BLOCKS - 1
                        and s == nslab - 1
                        and tail_split < slabs[s]
                    ):
                        pieces = [slabs[s] - tail_split, tail_split]
                    c0 = 0
                    for w in pieces:
                        src = x_ap[
                            r0 : r0 + TB * P,
                            slab_off[s] + c0 : slab_off[s] + c0 + w,
                        ].rearrange("(t p) c -> p t c", p=P)
                        dst = xh[:].rearrange("p (t c) -> p t c", t=TB)[
                            :, :, c0 : c0 + w
                        ]
                        nc.gpsimd.dma_start(dst, src)
                        c0 += w
                    big = xh
                    for t in range(TB):
                        xs[t][s] = big[:, t * slabs[s] : (t + 1) * slabs[s]]
                    return
                xt = xp.tile([P, TB * slabs[s]], f32, tag=f"x{s}")
                if combine_dma:
                    src = x_ap[
                        r0 : r0 + TB * P, slab_off[s] : slab_off[s] + slabs[s]
                    ].rearrange("(t p) c -> p t c", p=P)
                    dst = xt[:].rearrange("p (t c) -> p t c", t=TB)
                    eng.dma_start(dst, src)
                else:
                    for t in range(TB):
                        eng.dma_start(
                            xt[:, t * slabs[s] : (t + 1) * slabs[s]],
                            x_ap[
                                r0 + t * P : r0 + (t + 1) * P,
                                slab_off[s] : slab_off[s] + slabs[s],
                            ],
                        )
                if use_fp16 and stage >= 1:
                    # fp16 copy on DVE (2x single-src mode) so the PE
                    # transpose runs at 1 cyc/row instead of fp32's 2.
                    xh = xc.tile([P, TB * slabs[s]], fp16, tag=f"xc{s}")
                    nc.vector.tensor_copy(xh[:], xt[:])
                    big = xh
                else:
                    big = xt
                for t in range(TB):
                    xs[t][s] = big[:, t * slabs[s] : (t + 1) * slabs[s]]

            def finish_block(b, acc):
                ob = obuf.tile([1, NBLK], f32)
                if stage < 4:
                    nc.vector.memset(ob[:], 0.0)
                else:
                    nc.vector.tensor_scalar_add(ob[:], acc[:], g_sb[0:1, 0:1])
                # out stores ride a configurable queue (they depend on
                # compute; on the load rings they could stall later x loads).
                eng_by_name[store_eng].dma_start(
                    out_ap[0:1, b * NBLK : (b + 1) * NBLK], ob[:]
                )

            def one_pass():
                if not two_phase:
                    for b in range(BLOCKS):
                        xs = [[None] * nslab for _ in range(TB)]
                        for s in range(nslab):
                            load_slab(b, s, xs)
                        acc = psa.tile([1, NBLK], f32)
                        for c in range(KC):
                            do_chunk(b, c, xs, acc)
                        finish_block(b, acc)
                    return
                # Two-phase schedule: every block's slab-0 work first (all
                # BLOCKS accumulators stay live in PSUM), then slab-1 per
                # block. The post-last-DMA tail is one block's last-slab
                # chunks only.
                xs_all = [[[None] * nslab for _ in range(TB)] for _ in range(BLOCKS)]
                accs = [None] * BLOCKS
                if pack_accs:
                    # three block accumulators share one PSUM bank at
                    # partitions 0/32/64 (the legal AP base partitions);
                    # the fourth gets its own bank. Frees 2 banks for pst.
                    acc3 = psa.tile([128, NBLK], f32, tag="acc3")
                    accl = psa.tile([1, NBLK], f32, tag="accl")
                    accs = [acc3[32 * b : 32 * b + 1, :] for b in range(3)] + [accl]
                first_chunks = [c for c in range(KC) if chunk_slab[c][0] < nslab - 1]
                last_chunks = [c for c in range(KC) if chunk_slab[c][0] == nslab - 1]
                for b in range(BLOCKS):
                    for s in range(nslab - 1):
                        load_slab(b, s, xs_all[b])
                    if not pack_accs:
                        acc = psa.tile([1, NBLK], f32)
                        accs[b] = acc
                    for c in first_chunks:
                        do_chunk(b, c, xs_all[b], accs[b])
                s_last = nslab - 1
                for b in range(BLOCKS):
                    load_slab(b, s_last, xs_all[b])
                    for c in last_chunks:
                        do_chunk(b, c, xs_all[b], accs[b])
                    ob = obuf.tile([1, NBLK], f32)
                    if stage < 4:
                        nc.vector.memset(ob[:], 0.0)
                    else:
                        nc.vector.tensor_scalar_add(ob[:], accs[b][:], g_sb[0:1, 0:1])
                    # out stores ride a configurable queue (they depend on
                    # compute; on the load rings they could stall x loads).
                    eng_by_name[store_eng].dma_start(
                        out_ap[0:1, b * NBLK : (b + 1) * NBLK], ob[:]
                    )

            if loop_repeat:
                # HW loop wrapper for differential timing: each iteration is
                # a full cold-ish pass (the back-edge is an all-engine drain).
                with tc.For_i(
                    0, loop_repeat, 1, hint_engines=(mybir.EngineType.PE,)
                ):
                    one_pass()
            else:
                for _rep in range(repeat):
                    one_pass()

    nc.compile()
    return nc


def make_in_maps(x, raw_params, edges):
    x = np.ascontiguousarray(np.asarray(x, dtype=np.float32))
    raw_params = np.asarray(raw_params, dtype=np.float64)
    edges = np.asarray(edges)
    assert x.shape == (N, D), x.shape

    # Tiny host-side coefficient math (O(D); the O(N*D) pass runs on device).
    means = np.tanh(raw_params[:D]) * 2.0
    stds = np.logaddexp(0.0, raw_params[D:]) + 1e-6  # softplus + eps
    deg = np.zeros(D, dtype=np.float64)
    np.add.at(deg, edges.reshape(-1), 1.0)
    p = np.sqrt(0.5 * deg) / stds
    q = -means * p
    gamma = float(-np.sum(deg * (np.log(stds) + HALF_LOG_2PI)))

    p2d = p.reshape(KC, P).T.astype(np.float32)
    q2d = q.reshape(KC, P).T.astype(np.float32)
    coefs = np.zeros((P, 2 * KC + 1 + P), dtype=np.float32)
    coefs[:, 0:KC] = p2d
    coefs[:, KC : 2 * KC] = q2d
    coefs[:, 2 * KC] = gamma
    coefs[:, 2 * KC + 1 :] = np.eye(P, dtype=np.float32)

    in_maps = []
    for c in range(NCORES):
        shard = x[c * NSHARD : (c + 1) * NSHARD]
        m = {"x": shard, "coefs": coefs}
        if DEFAULT_CFG.get("use_fp16"):
            m["identh"] = np.eye(P, dtype=np.float16)
        in_maps.append(m)
    return in_maps


def kernel(x, raw_params, edges, _trace=False):
    global _PROGRAM, LAST_RESULT
    in_maps = make_in_maps(x, raw_params, edges)

    if _PROGRAM is None:
        _PROGRAM = _build_program(**DEFAULT_CFG)
    nc = _PROGRAM

    LAST_RESULT = run_bass_kernel_spmd(
        nc, in_maps, core_ids=list(range(NCORES)), trace=_trace
    )
    out = np.concatenate(
        [LAST_RESULT.results[c]["out"].reshape(-1) for c in range(NCORES)]
    )
    return out.astype(np.float32)

